# revision 1
# baseline (speedup 1.0000x reference)
"""Joint Maximum Mean Discrepancy loss on 8 Trainium2 NeuronCores.

Math: for streams (s0,t0) and (s1,t1), the reference builds per-stream
Gaussian kernels K_r = exp(-gamma_r * dist_r) over feats_r = [src; tgt]
(N=8192 rows), takes their elementwise product, and returns
mean(s2s + t2t - 2*s2t) over the B x B blocks.

Device decomposition:
  exponent E_ij = 2*(W @ W.T)_ij - c_i - c_j
  with W = [sqrt(g0)*X0, sqrt(g1)*X1] (N x 320), c_i = g0*|X0_i|^2 +
  g1*|X1_i|^2, and gamma_r from the closed form
  sum(dist_r) = 2*N*sum(sq_r) - 2*||colsum(X_r)||^2. The joint kernel is
  exp(E) in a single matmul + exp; -c_i and -c_j ride as two extra
  contraction rows (K = 322). Symmetry of E halves the work via a
  block-cyclic cover: core k owns row-chunks {k, k+8} (chunk = 512 rows)
  and computes 17 [512 x 512] blocks — column offsets d=0..8 from row
  chunk k, d=0..7 from row chunk k+8 — counting every unordered
  off-diagonal chunk pair exactly once (weight 2) and diagonals once
  (weight 1). Per-block sums (fp32, one per PSUM partition) return to the
  host, which applies weights/signs and the final reduction in float64.

Per-core device program (SPMD — identical instructions, data differs):
  - lhs  [2, 322, 512]  : [2*W_rows.T ; ones ; -c_rows] for chunks k, k+8
  - rhs  [16, 322, 512] : [W_cols.T ; -c_cols ; ones], chunk-major with
    chunk order rotated by k so the program's chunk index d is core-local
  - 17 blocks x 4 m-tiles: 3 matmuls (K chunks 128/128/66) into PSUM
    [128, 2048] (4 banks), one Exp activation over the 4 banks with
    accum_out producing the per-partition block sum
  - out "acc" [128, 17] fp32
"""

import os

import numpy as np

import concourse.bacc as bacc
import concourse.bass as bass
import concourse.mybir as mybir
import concourse.tile as tile
from concourse.bass_utils import run_bass_kernel_spmd

B = 4096
D0, D1 = 256, 64
N = 2 * B
CH = 512          # rows per chunk
NCHUNK = 16
NCORE = 8
KF = D0 + D1      # feature contraction rows
KT = KF + 2       # + ones row + (-c) row
KQ = [(0, 128), (128, 256), (256, KT)]   # contraction partition chunks
MT = 128          # m-tile rows
NMT = CH // MT    # m-tiles per row-chunk (4)
NBLK = 17         # blocks per core (9 from chunk k, 8 from chunk k+8)
NCOL = NBLK       # acc columns

# bf16 streams the PE at 1 cyc/row (f32r: 2, f32: 4); measured end loss
# error vs float64 is ~1.5e-4 rel — far inside the fp32-envelope budget.
_DT_NAME = os.environ.get("JMMD_MM_DTYPE", "bf16")
_DT = {
    "f32r": mybir.dt.float32r,
    "f32": mybir.dt.float32,
    "bf16": mybir.dt.bfloat16,
}[_DT_NAME]
_N_WARMUP = int(os.environ.get("JMMD_WARMUP", "28"))

LAST_EXEC_NS = None
LAST_RESULTS = None

_CACHE: dict = {}


def _np_dtype():
    if _DT_NAME == "bf16":
        import ml_dtypes

        return ml_dtypes.bfloat16
    return np.float32


def _build():
    if "nc" in _CACHE:
        return _CACHE["nc"]
    nc = bacc.Bacc(
        "TRN2", target_bir_lowering=False, debug=False, enable_asserts=False
    )
    f32 = mybir.dt.float32
    lhs_dram = nc.dram_tensor("lhs", [2, KT, CH], _DT, kind="ExternalInput").ap()
    rhs_dram = nc.dram_tensor("rhs", [NCHUNK, KT, CH], _DT, kind="ExternalInput").ap()
    acc_dram = nc.dram_tensor("acc", [MT, NCOL], f32, kind="ExternalOutput").ap()

    with tile.TileContext(nc) as tc:
        with (
            tc.tile_pool(name="const", bufs=1) as const,
            tc.tile_pool(name="psum", bufs=2, space=bass.MemorySpace.PSUM) as psum,
        ):
            lhs_t = {}
            rhs_t = {}

            def load_lhs(g):
                for q, (a, b) in enumerate(KQ):
                    t = const.tile([b - a, CH], _DT, tag=f"lhs{g}_{q}")
                    nc.sync.dma_start(t[:], lhs_dram[g, a:b, :])
                    lhs_t[(g, q)] = t

            def load_rhs(ch, eng):
                for q, (a, b) in enumerate(KQ):
                    t = const.tile([b - a, CH], _DT, tag=f"rhs{q}_{ch}")
                    eng.dma_start(t[:], rhs_dram[ch, a:b, :])
                    rhs_t[(q, ch)] = t

            # warmup scratch memset goes FIRST on gpsimd — anything queued
            # behind bulk DMAs on that engine would stall the PE program.
            scratch = None
            if _N_WARMUP:
                scratch = const.tile([MT, 256], _DT, tag="warm_src")
                nc.gpsimd.memset(scratch[:], 0.0)

            # block 0's operands race down both DMA engines in parallel;
            # lhsB is not needed until block 9 (~t+23us)
            load_lhs(0)
            load_rhs(0, nc.gpsimd)
            for ch in (1, 3, 5):
                load_rhs(ch, nc.sync)
            for ch in (2, 4, 6):
                load_rhs(ch, nc.gpsimd)
            load_lhs(1)
            for ch in (7, 9, 11, 13, 15):
                load_rhs(ch, nc.sync)
            for ch in (8, 10, 12, 14):
                load_rhs(ch, nc.gpsimd)

            acc_t = const.tile([MT, NCOL], f32, tag="acc")

            # HAM warmup: dense dummy matmuls while input DMAs stream, so
            # real matmuls start at the warm PE clock.
            if _N_WARMUP:
                warm_ps = psum.tile([MT, NMT * CH], f32, tag="ps")
                for _ in range(_N_WARMUP):
                    nc.tensor.matmul(
                        warm_ps[:, :MT],
                        scratch[:, :MT],
                        scratch[:, MT:],
                        start=True,
                        stop=True,
                    )

            for g, nd in ((0, 9), (1, 8)):
                for d in range(nd):
                    ch = d if g == 0 else 8 + d
                    col = d if g == 0 else 9 + d
                    ps = psum.tile([MT, NMT * CH], f32, tag="ps")
                    for m in range(NMT):
                        for q in range(3):
                            nc.tensor.matmul(
                                ps[:, m * CH:(m + 1) * CH],
                                lhs_t[(g, q)][:, m * MT:(m + 1) * MT],
                                rhs_t[(q, ch)][:],
                                start=(q == 0),
                                stop=(q == 2),
                            )
                    nc.scalar.activation(
                        ps[:],
                        ps[:],
                        mybir.ActivationFunctionType.Exp,
                        accum_out=acc_t[:, col:col + 1],
                    )
            nc.sync.dma_start(acc_dram[:], acc_t[:])
    nc.compile()
    _CACHE["nc"] = nc
    return nc


def _pack_inputs(s0, s1, t0, t1):
    X0 = np.concatenate([s0, t0], axis=0).astype(np.float64)
    X1 = np.concatenate([s1, t1], axis=0).astype(np.float64)

    def gamma_of(X):
        sq = np.sum(X * X, axis=1)
        sdist = 2.0 * X.shape[0] * np.sum(sq) - 2.0 * np.sum(np.sum(X, axis=0) ** 2)
        return (X.shape[0] ** 2 - X.shape[0]) / sdist, sq

    g0, sq0 = gamma_of(X0)
    g1, sq1 = gamma_of(X1)
    c = g0 * sq0 + g1 * sq1
    W = np.concatenate([np.sqrt(g0) * X0, np.sqrt(g1) * X1], axis=1)  # [N, 320]
    npdt = _np_dtype()

    # chunk-major staging of [W.T ; -c ; ones] so every device DMA reads a
    # contiguous range
    Wt = np.empty((NCHUNK, KT, CH), dtype=np.float64)
    for ch in range(NCHUNK):
        rows = slice(ch * CH, (ch + 1) * CH)
        Wt[ch, :KF] = W[rows].T
        Wt[ch, KF] = -c[rows]
        Wt[ch, KF + 1] = 1.0
    Wt = Wt.astype(npdt)

    def lhs_for(chunk):
        rows = slice(chunk * CH, (chunk + 1) * CH)
        out = np.empty((KT, CH), dtype=np.float64)
        out[:KF] = 2.0 * W[rows].T
        out[KF] = 1.0
        out[KF + 1] = -c[rows]
        return out.astype(npdt)

    in_maps = []
    for k in range(NCORE):
        lhs = np.stack([lhs_for(k), lhs_for((k + 8) % NCHUNK)])
        rhs = Wt[[(k + d) % NCHUNK for d in range(NCHUNK)]]
        in_maps.append({"lhs": lhs, "rhs": np.ascontiguousarray(rhs)})
    return in_maps


def _combine(results):
    sgn = lambda ch: 1.0 if ch < NCHUNK // 2 else -1.0
    total = 0.0
    for k in range(NCORE):
        acc = np.asarray(results[k]["acc"], dtype=np.float64)  # [128, 17]
        colsum = acc.sum(axis=0)
        for col in range(NCOL):
            if col < 9:
                d, row_chunk = col, k
            else:
                d, row_chunk = col - 9, (k + 8) % NCHUNK
            col_chunk = (row_chunk + d) % NCHUNK
            w = 1.0 if d == 0 else 2.0
            s = sgn(row_chunk) * sgn(col_chunk)
            total += w * s * colsum[col]
    return total / (B * B)


def kernel(s0, s1, t0, t1):
    global LAST_EXEC_NS, LAST_RESULTS
    nc = _build()
    in_maps = _pack_inputs(
        np.asarray(s0), np.asarray(s1), np.asarray(t0), np.asarray(t1)
    )
    trace = os.environ.get("JMMD_TRACE", "0") == "1"
    res = run_bass_kernel_spmd(nc, in_maps, core_ids=list(range(NCORE)), trace=trace)
    LAST_EXEC_NS = res.exec_time_ns
    LAST_RESULTS = res
    return np.float32(_combine(res.results))

